# revision 71
# baseline (speedup 1.0000x reference)
"""Trainium2 Bass kernel for LocalWindowAttention.

Model (reference): B=2, S=4096, D=1024, H=16 heads, hd=64, window W=16
(8 left, 7 right), four dim->dim projections (q/k/v/out, torch-Linear
convention y = x @ W.T), per-token windowed softmax attention.

Sharding: 8 cores = 2 batches x 4 sequence chunks of 1024 tokens.  Each
core receives a zero-padded halo of 8 left / 7 right tokens (1039 total)
so K/V at chunk boundaries are computed locally - no collectives.

Design ("W", half-stacked 128-exact key windows):
  Per 128-token q block b, the two 64-token halves use 128-key windows
  [128b, 128b+128) and [128b+64, 128b+192) in halo coords, so every
  score tile is a dense [128, 128]: rows = both halves stacked (row p =
  token 128b+p), cols = window-local keys j with in-band iff
  j - (p % 64) in [0, 16).
  - scores: 2 matmuls per head (one per half, 79-key streams - keys past
    78 are never in-band), 4 same-parity heads per PSUM bank.
  - exp: one ScalarE activation per 4-head group, strided into
    ES [128, 16 head slots, 128] fp16; cols 79:128 stay zero from a
    one-time ring memset.
  - band mask as 0/1 MULTIPLY (DVE, middle-dim broadcast keeps 2x mode).
  - denominators: DVE row-reduce (fp16) + subtract static pad count
    (adj); halo-pad keys give exp(0)=1 which adj removes exactly.
  - 1/denom multiply on GpSimd (Pool) - otherwise-idle engine.
  - probs transpose via DMA xbar transpose (dma_start_transpose), one
    per 8-head half: pT[k, h, q] = ES[q, h, k]; no PE transposes, no
    PSUM evacuation copies.
  - AV: per head 2 matmuls (halves), stationary v tiles aligned to the
    two window grids: v_sb (128-aligned) and v2 (64-shifted copy made
    by SBUF->SBUF DMA); 4 head-pairs share an av PSUM bank so ScalarE
    evacuates each bank with one wide copy.
  out-proj streams attnT against Wo.T; PSUM evacuated fp16 by ScalarE,
  output DMA'd fp16 (host upcasts to fp32).

Scheduling notes (tuned against the TimelineSim cost model):
  - software pipelining: scores/softmax of block b+6 are emitted before
    AV of block b, so the in-order PE queue never waits out the softmax
    chain; ES/pT rings are sized so buffer-reuse WAR waits are trivial.
  - every engine sequencer is in-order and DMA completions gate queue
    reuse, so DMAs are spread across the SP HWDGE queue (input loads,
    xbars), the Activation HWDGE queue (v2 shift copies, last-block
    outputs) and the GpSimd SWDGE queue (x slices, per-block outputs).
  - startup: the q projection runs k-outer in 3-bank groups and the
    first-needed halves of wq/x ship first, so matmuls start ~3us in.
  - the last block's out-projection is split into 256-wide chunks to
    shrink the end-of-kernel drain.
"""

import numpy as np

import concourse.bass as bass
import concourse.mybir as mybir
import concourse.tile as tile
from concourse import bacc
from concourse.bass_utils import run_bass_kernel_spmd

F16 = mybir.dt.float16
F32 = mybir.dt.float32

B, S, D = 2, 4096, 1024
H, HD = 16, 64
WIN, LP, RP = 16, 8, 7
NCORES = 8
CHUNK = S // 4            # tokens per core
TH = CHUNK + LP + RP      # real halo token count (1039)
THP = 1152                # padded halo (9*128) for kT / v key windows
NB = CHUNK // 128         # q blocks per core (8)
DT = D // 128             # 128-row tiles across D (8)
NVT = THP // 128          # v token tiles (9; last has 15 real rows)
VTAIL = TH - 128 * (NVT - 1)  # 15

TRACE = False             # test.py may set kernel.TRACE = True
LAST_RESULTS = None       # BassKernelResults of the most recent run

_PROGRAM = None


def _build_program():
    nc = bacc.Bacc("TRN2", target_bir_lowering=False, debug=False)

    xT_d = nc.dram_tensor("xT", [D, TH], F16, kind="ExternalInput")
    wq_d = nc.dram_tensor("wqT", [D, D], F16, kind="ExternalInput")
    wk_d = nc.dram_tensor("wkT", [D, D], F16, kind="ExternalInput")
    wv_d = nc.dram_tensor("wvT", [D, D], F16, kind="ExternalInput")
    wo_d = nc.dram_tensor("woT", [D, D], F16, kind="ExternalInput")
    adj_d = nc.dram_tensor("adj", [128, NB], F32, kind="ExternalInput")
    band_d = nc.dram_tensor("band01", [128, 128], F16, kind="ExternalInput")
    out_d = nc.dram_tensor("out", [CHUNK, D], F16, kind="ExternalOutput")

    with tile.TileContext(nc) as tc:
        with (
            tc.tile_pool(name="const", bufs=1) as cpool,
            tc.tile_pool(name="acts", bufs=1) as apool,
            tc.tile_pool(name="wstream", bufs=2 * DT) as wpool,
            tc.tile_pool(name="soft", bufs=8) as spool,
            tc.tile_pool(name="outsb", bufs=5) as opool,
            tc.tile_pool(name="proj_ps", bufs=3, space="PSUM") as proj_ps,
            tc.tile_pool(name="score_ps", bufs=2, space="PSUM") as score_ps,
            tc.tile_pool(name="av_ps", bufs=3, space="PSUM") as av_ps,
        ):
            xT = apool.tile([128, DT, TH], F16)
            qT = apool.tile([128, DT, CHUNK], F16)
            kT = apool.tile([128, DT, THP], F16)
            v_sb = apool.tile([128, NVT, D], F16)
            v2 = apool.tile([128, NVT - 1, D], F16)
            attnT = apool.tile([128, DT, CHUNK], F16)

            # interleave wq tiles and xT slices so the k-outer first
            # projection can start after the first (wq, x) pair lands
            wq = []
            wsrcs, xsrcs = [], []
            for k in range(DT):
                wt = wpool.tile([128, D], F16, tag="w", name=f"wq_{k}")
                wsrc = wq_d.ap().rearrange("(j p) o -> p j o", p=128)[:, k]
                xsrc = xT_d.ap().rearrange("(j p) t -> p j t", p=128)[:, k]
                # pass-1 of the k-outer projection needs only wq[:, 0:384]
                # and x tokens < LP+512: ship those first at compute rate
                nc.sync.dma_start(wt[:, 0:384], wsrc[:, 0:384])
                nc.gpsimd.dma_start(xT[:, k, 0:LP + 512], xsrc[:, 0:LP + 512])
                wq.append(wt)
                wsrcs.append(wsrc)
                xsrcs.append(xsrc)
            for k in range(DT):
                nc.sync.dma_start(wq[k][:, 384:D], wsrcs[k][:, 384:D])
            for k in range(DT):
                nc.gpsimd.dma_start(xT[:, k, LP + 512:], xsrcs[k][:, LP + 512:])

            band01 = cpool.tile([128, 128], F16)
            nc.gpsimd.dma_start(band01, band_d.ap())
            adj_sb = cpool.tile([128, NB], F32)
            nc.gpsimd.dma_start(adj_sb, adj_d.ap())

            def load_w(dram, nm):
                tiles = []
                for k in range(DT):
                    wt = wpool.tile([128, D], F16, tag="w", name=f"{nm}_{k}")
                    nc.sync.dma_start(
                        wt, dram.ap().rearrange("(j p) o -> p j o", p=128)[:, k]
                    )
                    tiles.append(wt)
                return tiles

            evac_n = [0]

            def evac(dst, src):
                # alternate PSUM evacuation between DVE and ScalarE
                if evac_n[0] % 2 == 0:
                    nc.vector.tensor_copy(dst, src)
                else:
                    nc.scalar.activation(
                        dst, src, mybir.ActivationFunctionType.Copy
                    )
                evac_n[0] += 1

            # ---- qT projection, k-outer in 3-bank groups ----
            for c0 in (0, 512):
                for ms in ((0, 1, 2), (3, 4, 5), (6, 7)):
                    pss = [
                        proj_ps.tile([128, 512], F32, tag="proj",
                                     name=f"qp_{c0}_{m}")
                        for m in ms
                    ]
                    for k in range(DT):
                        for mi, m in enumerate(ms):
                            nc.tensor.matmul(
                                pss[mi],
                                wq[k][:, m * 128:(m + 1) * 128],
                                xT[:, k, LP + c0: LP + c0 + 512],
                                start=(k == 0),
                                stop=(k == DT - 1),
                            )
                    for mi, m in enumerate(ms):
                        evac(qT[:, m, c0:c0 + 512], pss[mi])

            # ---- kT projection (m-outer; tail cols memset) ----
            nc.gpsimd.memset(kT[:, :, TH:THP], 0)
            wk = load_w(wk_d, "wk")
            for m in range(DT):
                for (c0, cn) in ((0, 512), (512, 512), (1024, TH - 1024)):
                    ps = proj_ps.tile([128, 512], F32, tag="proj")
                    for k in range(DT):
                        nc.tensor.matmul(
                            ps[:, :cn],
                            wk[k][:, m * 128:(m + 1) * 128],
                            xT[:, k, c0:c0 + cn],
                            start=(k == 0),
                            stop=(k == DT - 1),
                        )
                    evac(kT[:, m, c0:c0 + cn], ps[:, :cn])

            # ---- v projection (natural layout) + 64-shifted copy ----
            nc.gpsimd.memset(v_sb[:, NVT - 1, :], 0)
            wv = load_w(wv_d, "wv")
            for j in range(NVT):
                rows = 128 if j < NVT - 1 else VTAIL
                for n in range(2):
                    ps = proj_ps.tile([128, 512], F32, tag="proj")
                    for k in range(DT):
                        nc.tensor.matmul(
                            ps[:rows, :],
                            xT[:, k, j * 128: j * 128 + rows],
                            wv[k][:, n * 512:(n + 1) * 512],
                            start=(k == 0),
                            stop=(k == DT - 1),
                        )
                    evac(v_sb[:rows, j, n * 512:(n + 1) * 512], ps[:rows, :])
                if j >= 1:
                    nc.scalar.dma_start(v2[0:64, j - 1, :], v_sb[64:128, j - 1, :])
                    nc.scalar.dma_start(v2[64:128, j - 1, :], v_sb[0:64, j, :])

            wo = load_w(wo_d, "wo")

            # ---- attention blocks (software-pipelined: scores/softmax of
            # block b+LOOKAHEAD are emitted before AV of block b so the PE,
            # which executes in order, never waits out the softmax chain) ----
            pT_tiles = {}
            es_boot = []
            for i in range(8):
                est = spool.tile([128, H, 128], F16, tag="es", name=f"es_boot{i}")
                nc.gpsimd.memset(est, 0)
                es_boot.append(est)
            del es_boot

            def scores_softmax(b):
                ES = spool.tile([128, H, 128], F16, tag="es")
                sums = spool.tile([128, H], F16, tag="sums")
                denom = spool.tile([128, H], F32, tag="denom")
                rinv = spool.tile([128, H], F32, tag="rinv")
                pT = spool.tile([128, H, 128], F16, tag="pt")

                # scores + per-group softmax chain; ES slot = head index.
                # Group (l, g) covers heads l+8g+2i (i=0..3, strided slices);
                # after both groups of a half (heads 8g:8g+8) finish, one
                # xbar transposes that contiguous half so AV can start.
                for gi, (l, g) in enumerate(((0, 0), (1, 0), (0, 1), (1, 1))):
                    e0 = l + 8 * g
                    if gi < 2:
                        sc = score_ps.tile([128, 4, 128], F32, tag="sc")
                    elif gi == 2:
                        psf = proj_ps.tile([128, 512], F32, tag="proj")
                        sc = psf.rearrange("p (i c) -> p i c", i=4)
                    else:
                        sc = av_ps.tile([128, 4, 128], F32, tag="av")
                    for i in range(4):
                        h = l + 8 * g + 2 * i
                        for s2 in (0, 1):
                            nc.tensor.matmul(
                                sc[64 * s2:64 * s2 + 64, i, 0:79],
                                qT[64 * l:64 * l + 64, h // 2,
                                   128 * b + 64 * s2: 128 * b + 64 * s2 + 64],
                                kT[64 * l:64 * l + 64, h // 2,
                                   128 * b + 64 * s2: 128 * b + 64 * s2 + 79],
                                start=True,
                                stop=True,
                            )
                    ESg = ES[:, e0:e0 + 7:2, :]
                    nc.scalar.activation(
                        ESg[:, :, 0:79], sc[:, :, 0:79],
                        mybir.ActivationFunctionType.Exp, scale=0.125,
                    )
                    # band mask (0/1 multiply; middle broadcast keeps 2x)
                    nc.vector.tensor_tensor(
                        ESg[:, :, 0:79],
                        ESg[:, :, 0:79],
                        band01[:, None, 0:79].broadcast_to([128, 4, 79]),
                        mybir.AluOpType.mult,
                    )
                    # denominator = row sum - static pad count
                    with nc.allow_low_precision("fp16 softmax sums"):
                        nc.vector.tensor_reduce(
                            sums[:, e0:e0 + 7:2], ESg[:, :, 0:79],
                            mybir.AxisListType.X, mybir.AluOpType.add,
                        )
                    nc.vector.tensor_tensor(
                        denom[:, e0:e0 + 7:2],
                        sums[:, e0:e0 + 7:2],
                        adj_sb[:, b:b + 1].broadcast_to([128, 4]),
                        mybir.AluOpType.subtract,
                    )
                    nc.vector.reciprocal(
                        rinv[:, e0:e0 + 7:2], denom[:, e0:e0 + 7:2]
                    )
                    # normalize on the otherwise-idle GpSimd engine
                    nc.gpsimd.tensor_tensor(
                        ESg[:, :, 0:79],
                        ESg[:, :, 0:79],
                        rinv[:, e0:e0 + 7:2, None].broadcast_to([128, 4, 79]),
                        mybir.AluOpType.mult,
                    )
                    if l == 1:
                        # both parities of this half done: transpose the
                        # contiguous half; pT[k, h, q] = ES[q, h, k]
                        nc.sync.dma_start_transpose(
                            pT[:, 8 * g:8 * g + 8, :], ES[:, 8 * g:8 * g + 8, :]
                        )
                pT_tiles[b] = pT

            def av_outproj(b):
                pT = pT_tiles.pop(b)
                # AV: per head one matmul per half-window; 4 head pairs share
                # a PSUM bank so evacuation is one wide DVE copy per bank
                for jg in (0, 1):
                    av = av_ps.tile([128, 4, 128], F32, tag="av")
                    for jj in range(4):
                        j = 4 * jg + jj
                        for li in (0, 1):
                            h = 2 * j + li
                            nc.tensor.matmul(
                                av[64 * li:64 * li + 64, jj, 0:64],
                                v_sb[:, b, 64 * h:64 * h + 64],
                                pT[:, h, 0:64],
                                start=True,
                                stop=True,
                            )
                            nc.tensor.matmul(
                                av[64 * li:64 * li + 64, jj, 64:128],
                                v2[:, b, 64 * h:64 * h + 64],
                                pT[:, h, 64:128],
                                start=True,
                                stop=True,
                            )
                    dst = attnT[:, 4 * jg:4 * jg + 4, 128 * b:128 * b + 128]
                    nc.scalar.activation(
                        dst, av, mybir.ActivationFunctionType.Copy
                    )

                # out-projection for this block; one merged output DMA.
                # Last block: 256-wide chunks, each evac'd and DMA'd as soon
                # as its matmuls finish, to shrink the end-of-kernel drain.
                osb = opool.tile([128, D], F16, tag="osb")
                if b < NB - 1:
                    for n in (0, 1):
                        ps = proj_ps.tile([128, 512], F32, tag="proj")
                        for k in range(DT):
                            nc.tensor.matmul(
                                ps,
                                attnT[:, k, 128 * b:128 * b + 128],
                                wo[k][:, n * 512:(n + 1) * 512],
                                start=(k == 0),
                                stop=(k == DT - 1),
                            )
                        dst = osb[:, n * 512:(n + 1) * 512]
                        nc.scalar.activation(
                            dst, ps, mybir.ActivationFunctionType.Copy
                        )
                    nc.gpsimd.dma_start(
                        out_d.ap()[b * 128:(b + 1) * 128, :], osb
                    )
                else:
                    for n in range(4):
                        psf = proj_ps.tile([128, 512], F32, tag="proj")
                        ps = psf[:, 0:256]
                        for k in range(DT):
                            nc.tensor.matmul(
                                ps,
                                attnT[:, k, 128 * b:128 * b + 128],
                                wo[k][:, n * 256:(n + 1) * 256],
                                start=(k == 0),
                                stop=(k == DT - 1),
                            )
                        dst = osb[:, n * 256:(n + 1) * 256]
                        if n % 2 == 1:
                            nc.vector.tensor_copy(dst, ps)
                        else:
                            nc.scalar.activation(
                                dst, ps, mybir.ActivationFunctionType.Copy
                            )
                        eng = nc.sync if n % 2 == 1 else nc.scalar
                        eng.dma_start(
                            out_d.ap()[b * 128:(b + 1) * 128,
                                       n * 256:(n + 1) * 256],
                            dst,
                        )

            LOOKAHEAD = 5
            for b in range(min(LOOKAHEAD, NB)):
                scores_softmax(b)
            for b in range(NB):
                av_outproj(b)
                if b + LOOKAHEAD < NB:
                    scores_softmax(b + LOOKAHEAD)

    nc.compile()
    return nc


def _get_program():
    global _PROGRAM
    if _PROGRAM is None:
        _PROGRAM = _build_program()
    return _PROGRAM


def _host_inputs(x, Wq, Wk, Wv, Wo):
    """Shard + preprocess full inputs into per-core input maps."""
    x = np.asarray(x, dtype=np.float32)
    wts = {}
    for name, w in (("wqT", Wq), ("wkT", Wk), ("wvT", Wv), ("woT", Wo)):
        wts[name] = np.ascontiguousarray(np.asarray(w, np.float32).T).astype(
            np.float16
        )

    # band01[p, j] = 1 iff window-local key j is in-band for stacked row p
    pp = np.arange(128)[:, None] % 64
    jj = np.arange(128)[None, :]
    band = (((jj - pp) >= 0) & ((jj - pp) <= WIN - 1)).astype(np.float16)

    in_maps = []
    for c in range(NCORES):
        bb, chunk = divmod(c, 4)
        g0 = chunk * CHUNK
        lo, hi = g0 - LP, g0 + CHUNK + RP
        xpad = np.zeros((TH, D), np.float32)
        src_lo, src_hi = max(lo, 0), min(hi, S)
        xpad[src_lo - lo: src_hi - lo] = x[bb, src_lo:src_hi]
        xT = np.ascontiguousarray(xpad.T).astype(np.float16)

        # adj[p, b] = # in-band keys of global token g0+128b+p outside [0, S)
        glob = g0 + (np.arange(NB * 128)).reshape(NB, 128)
        pos = glob[:, :, None] - LP + np.arange(WIN)[None, None, :]
        counts = ((pos < 0) | (pos >= S)).sum(axis=2).astype(np.float32)
        adj = np.ascontiguousarray(counts.T)  # [128, NB]

        in_maps.append({"xT": xT, "adj": adj, "band01": band, **wts})
    return in_maps


def kernel(x, Wq, Wk, Wv, Wo):
    global LAST_RESULTS
    nc = _get_program()
    in_maps = _host_inputs(x, Wq, Wk, Wv, Wo)
    res = run_bass_kernel_spmd(
        nc, in_maps, core_ids=list(range(NCORES)), trace=TRACE
    )
    LAST_RESULTS = res
    out = np.empty((B, S, D), np.float32)
    for c in range(NCORES):
        bb, chunk = divmod(c, 4)
        out[bb, chunk * CHUNK:(chunk + 1) * CHUNK] = res.results[c][
            "out"
        ].astype(np.float32)
    return out
